# revision 1
# baseline (speedup 1.0000x reference)
"""Trainium2 Bass kernel for MF embedding-lookup + dot-product scoring.

out[u, i] = dot(user_hiddens[user_ids[u]], item_hiddens[item_ids[i]])

Sharding: 2D over 8 cores — 4 user groups (1024 users) x 2 item groups
(2048 items); tables replicated to every core's HBM. Per core:
  - indirect-DMA gathers 128 rows/call (one index per partition), 8 user
    calls + 16 item calls
  - PE transpose to [64, batch]; split each f32 value into bf16 hi+lo
  - per item tile: 3-term bf16 matmuls (hi*hi + hi*lo + lo*hi) accumulate
    in f32 PSUM -> ~1e-5 rel err at ~4x the fp32 matmul speed
  - item tile stationary, users moving: the matmul for item tile t fires
    as soon as tile t's gather lands (no global barrier on the gathers)
  - output [2048 items, 1024 users] written in 512 KB contiguous chunks
Host transposes each core slab into the final [4096, 4096].
"""

import numpy as np

import concourse.bacc as bacc
import concourse.bass as bass
import concourse.mybir as mybir
import concourse.tile as tile
from concourse.bass_utils import run_bass_kernel_spmd
from concourse.masks import make_identity

NUM_USERS = 1_000_000
NUM_ITEMS = 100_000
D = 64
BU = 4096
BI = 4096
N_CORES = 8
RU = 4              # user groups
RI = 2              # item groups
UC = BU // RU       # users per core = 1024
IC = BI // RI       # items per core = 2048
P = 128
UT = UC // P        # user tiles per core = 8
IT = IC // P        # item tiles per core = 16
NBLK = 512          # matmul moving free dim
NH = UC // NBLK     # user halves per item tile = 2

_cache = {}


def _build():
    nc = bacc.Bacc()
    ut_dram = nc.dram_tensor(
        "user_table", [NUM_USERS, D], mybir.dt.float32, kind="ExternalInput"
    )
    it_dram = nc.dram_tensor(
        "item_table", [NUM_ITEMS, D], mybir.dt.float32, kind="ExternalInput"
    )
    uid_dram = nc.dram_tensor("uids", [P, UT], mybir.dt.int32, kind="ExternalInput")
    iid_dram = nc.dram_tensor("iids", [P, IT], mybir.dt.int32, kind="ExternalInput")
    out_dram = nc.dram_tensor(
        "out", [IC, UC], mybir.dt.float32, kind="ExternalOutput"
    )

    f32 = mybir.dt.float32
    bf16 = mybir.dt.bfloat16

    with tile.TileContext(nc) as tc:
        with (
            tc.tile_pool(name="const", bufs=1) as constp,
            tc.tile_pool(name="idx", bufs=1) as idxp,
            tc.tile_pool(name="gath", bufs=24) as gathp,
            tc.tile_pool(name="ops", bufs=1) as opsp,
            tc.tile_pool(name="vt", bufs=4) as vtp,
            tc.tile_pool(name="tp", bufs=2, space="PSUM") as tpp,
            tc.tile_pool(name="mm", bufs=3, space="PSUM") as mmp,
            tc.tile_pool(name="outp", bufs=3) as outp,
        ):
            ident = constp.tile([P, P], f32)
            make_identity(nc, ident[:])

            uids = idxp.tile([P, UT], mybir.dt.int32)
            iids = idxp.tile([P, IT], mybir.dt.int32)
            nc.sync.dma_start(out=uids[:], in_=uid_dram[:])
            nc.sync.dma_start(out=iids[:], in_=iid_dram[:])

            # --- user prologue: gather + transpose + bf16 hi/lo split ---
            # ustack: [uhi; uhi] duplicated across the two partition halves
            # so one K=128 matmul against [vhi; vlo] yields hi*hi + lo_v*hi_u.
            ustack = opsp.tile([2 * D, UC], bf16)
            ulo = opsp.tile([D, UC], bf16)
            for t in range(UT):
                g = gathp.tile([P, D], f32)
                nc.gpsimd.indirect_dma_start(
                    out=g[:],
                    out_offset=None,
                    in_=ut_dram[:],
                    in_offset=bass.IndirectOffsetOnAxis(
                        ap=uids[:, t : t + 1], axis=0
                    ),
                )
                ps = tpp.tile([D, P], f32)
                nc.tensor.transpose(ps[:], g[:], ident[:])
                sl = slice(t * P, (t + 1) * P)
                nc.scalar.copy(out=ustack[0:D, sl], in_=ps[:])
                nc.scalar.copy(out=ustack[D : 2 * D, sl], in_=ps[:])
                nc.vector.tensor_tensor(
                    out=ulo[:, sl],
                    in0=ps[:],
                    in1=ustack[0:D, sl],
                    op=mybir.AluOpType.subtract,
                )

            # --- item stream: gather -> transpose -> hi/lo -> matmuls -> out ---
            for t in range(IT):
                g = gathp.tile([P, D], f32)
                nc.gpsimd.indirect_dma_start(
                    out=g[:],
                    out_offset=None,
                    in_=it_dram[:],
                    in_offset=bass.IndirectOffsetOnAxis(
                        ap=iids[:, t : t + 1], axis=0
                    ),
                )
                ps = tpp.tile([D, P], f32)
                nc.tensor.transpose(ps[:], g[:], ident[:])
                # vstack = [vhi; vlo] on the two partition halves
                vstack = vtp.tile([2 * D, P], bf16)
                nc.scalar.copy(out=vstack[0:D, :], in_=ps[:])
                nc.vector.tensor_tensor(
                    out=vstack[D : 2 * D, :],
                    in0=ps[:],
                    in1=vstack[0:D, :],
                    op=mybir.AluOpType.subtract,
                )

                ot = outp.tile([P, UC], f32)
                po = mmp.tile([P, UC], f32)  # two PSUM banks
                for h in range(NH):
                    hs = slice(h * NBLK, (h + 1) * NBLK)
                    # terms hi_v*hi_u + lo_v*hi_u (K=128 stacked)
                    nc.tensor.matmul(
                        po[:, hs],
                        lhsT=vstack[:, :],
                        rhs=ustack[:, hs],
                        start=True,
                        stop=False,
                    )
                    # term hi_v*lo_u (K=64)
                    nc.tensor.matmul(
                        po[:, hs],
                        lhsT=vstack[0:D, :],
                        rhs=ulo[:, hs],
                        start=False,
                        stop=True,
                    )
                if t % 2 == 0:
                    nc.scalar.copy(out=ot[:], in_=po[:])
                else:
                    nc.vector.tensor_copy(out=ot[:], in_=po[:])
                nc.sync.dma_start(
                    out=out_dram[t * P : (t + 1) * P, :], in_=ot[:]
                )
    nc.finalize()
    return nc


def kernel(user_hiddens, item_hiddens, user_ids, item_ids, **_):
    user_hiddens = np.ascontiguousarray(user_hiddens, dtype=np.float32)
    item_hiddens = np.ascontiguousarray(item_hiddens, dtype=np.float32)
    user_ids = np.asarray(user_ids)
    item_ids = np.asarray(item_ids)

    if "nc" not in _cache:
        _cache["nc"] = _build()
    nc = _cache["nc"]

    in_maps = []
    for c in range(N_CORES):
        cu, ci = divmod(c, RI)
        uc = user_ids[cu * UC : (cu + 1) * UC]
        icd = item_ids[ci * IC : (ci + 1) * IC]
        # [P, T] transposed id layout: idx[p, t] = ids[t*128 + p]
        uids_t = np.ascontiguousarray(uc.astype(np.int32).reshape(UT, P).T)
        iids_t = np.ascontiguousarray(icd.astype(np.int32).reshape(IT, P).T)
        in_maps.append(
            {
                "user_table": user_hiddens,
                "item_table": item_hiddens,
                "uids": uids_t,
                "iids": iids_t,
            }
        )

    res = run_bass_kernel_spmd(nc, in_maps, list(range(N_CORES)))
    out = np.empty((BU, BI), dtype=np.float32)
    for c in range(N_CORES):
        cu, ci = divmod(c, RI)
        out[cu * UC : (cu + 1) * UC, ci * IC : (ci + 1) * IC] = res.results[c][
            "out"
        ].T
    return out



# revision 6
# speedup vs baseline: 1.0915x; 1.0915x over previous
"""Trainium2 Bass kernel for MF embedding-lookup + dot-product scoring.

out[u, i] = dot(user_hiddens[user_ids[u]], item_hiddens[item_ids[i]])

Sharding: 2D over 8 cores - 4 user groups (1024 users) x 2 item groups
(2048 items); tables replicated to every core's HBM. Per core:
  - users: 8 indirect-DMA gathers (128 f32 rows each) -> PE transpose ->
    bf16 cast -> uhi [64, 1024]
  - items: host converts the item table to bf16 with columns duplicated
    to 128 (256B rows) and range-buckets this core's item ids into 4
    static 25600-row table slices; 4 dma_gather(transpose=True) calls
    land v^T directly in SBUF as bf16 [128, n] - no PE transposes, no
    casts on the item side, and int16 indices stay in range.
  - per 128-item tile: 2 matmuls (K=64, N=512): lhsT = gathered v^T
    slice, rhs = uhi halves -> f32 PSUM
  - PSUM -> SBUF fp16 casts alternate vector/scalar; per-bucket batched
    DMAs write only the real (non-pad) rows -> out [2048, 1024] fp16
Host un-permutes the bucketed item order, transposes, upcasts to f32,
and assembles the full [4096, 4096].
"""

import numpy as np
import ml_dtypes

import concourse.bacc as bacc
import concourse.bass as bass
import concourse.mybir as mybir
import concourse.tile as tile
from concourse.bass_utils import run_bass_kernel_spmd
from concourse.masks import make_identity

NUM_USERS = 1_000_000
NUM_ITEMS = 100_000
D = 64
E = 128             # bf16 item row width (256B, dma_gather granularity)
BU = 4096
BI = 4096
N_CORES = 8
RU = 4              # user groups
RI = 2              # item groups
UC = BU // RU       # users per core = 1024
IC = BI // RI       # items per core = 2048
P = 128
UT = UC // P        # user gather calls = 8
NBLK = 512          # matmul moving free dim (one PSUM bank of f32)
NH = UC // NBLK     # user halves per item tile = 2
NBUCKET = 4
BWIDTH = 25_600     # static item-table range per bucket (< 32768)

_cache = {}


def _ceil128(n):
    return (n + P - 1) // P * P


def _build(mks, nks):
    """mks: per-bucket padded index counts (x128); nks: real counts."""
    nc = bacc.Bacc()
    ut_dram = nc.dram_tensor(
        "user_table", [NUM_USERS, D], mybir.dt.float32, kind="ExternalInput"
    )
    it_dram = nc.dram_tensor(
        "item_dup", [NUM_ITEMS, E], mybir.dt.bfloat16, kind="ExternalInput"
    )
    uid_dram = nc.dram_tensor("uids", [P, UT], mybir.dt.int32, kind="ExternalInput")
    icols = sum(mks) // 16
    iidx_dram = nc.dram_tensor(
        "iidx", [P, icols], mybir.dt.int16, kind="ExternalInput"
    )
    out_rows = sum(nks)
    out_dram = nc.dram_tensor(
        "out", [out_rows, UC], mybir.dt.float16, kind="ExternalOutput"
    )

    f32 = mybir.dt.float32
    bf16 = mybir.dt.bfloat16
    fp16 = mybir.dt.float16

    with tile.TileContext(nc) as tc:
        with (
            tc.tile_pool(name="const", bufs=1) as constp,
            tc.tile_pool(name="idx", bufs=1) as idxp,
            tc.tile_pool(name="gath", bufs=1) as gathp,
            tc.tile_pool(name="ops", bufs=1) as opsp,
            tc.tile_pool(name="tp", bufs=2, space="PSUM") as tpp,
            tc.tile_pool(name="mm", bufs=3, space="PSUM") as mmp,
            tc.tile_pool(name="outp", bufs=2) as outp,
        ):
            ident = constp.tile([P, P], f32)
            make_identity(nc, ident[:])

            uids = idxp.tile([P, UT], mybir.dt.int32)
            iidx = idxp.tile([P, icols], mybir.dt.int16)
            nc.sync.dma_start(out=uids[:], in_=uid_dram[:])
            nc.sync.dma_start(out=iidx[:], in_=iidx_dram[:])

            gu = gathp.tile([P, UT * D], f32)
            ctot = sum(mks)
            vmov = gathp.tile([P, ctot], bf16)

            def user_gather(t):
                nc.gpsimd.indirect_dma_start(
                    out=gu[:, t * D : (t + 1) * D],
                    out_offset=None,
                    in_=ut_dram[:],
                    in_offset=bass.IndirectOffsetOnAxis(
                        ap=uids[:, t : t + 1], axis=0
                    ),
                )

            def item_gather(k):
                coff = sum(mks[:k])
                icoff = coff // 16
                nc.gpsimd.dma_gather(
                    out_ap=vmov[:, coff : coff + mks[k]].rearrange(
                        "p (o n) -> p o n", o=1
                    ),
                    in_ap=it_dram[k * BWIDTH : min((k + 1) * BWIDTH, NUM_ITEMS), :],
                    idxs_ap=iidx[:, icoff : icoff + mks[k] // 16],
                    num_idxs=mks[k],
                    num_idxs_reg=mks[k],
                    elem_size=E,
                    transpose=True,
                )

            # gpsimd issue order: first half of users, first item bucket,
            # rest of users, remaining buckets
            for t in range(4):
                user_gather(t)
            item_gather(0)
            for t in range(4, UT):
                user_gather(t)
            for k in range(1, NBUCKET):
                item_gather(k)

            # --- user prologue: transpose + bf16 cast -> uhi [64, 1024] ---
            uhi = opsp.tile([D, UC], bf16)
            for t in range(UT):
                ps = tpp.tile([D, P], f32)
                nc.tensor.transpose(ps[:], gu[:, t * D : (t + 1) * D], ident[:])
                nc.vector.tensor_copy(
                    out=uhi[:, t * P : (t + 1) * P], in_=ps[:]
                )

            # --- item stream: matmuls straight off the gathered v^T ---
            cp = 0  # copy-engine rotation counter
            for k in range(NBUCKET):
                coff = sum(mks[:k])
                tk = mks[k] // P          # tiles in this bucket
                fk = nks[k] // P          # full tiles
                rk = nks[k] % P           # real rows in partial tile
                rowoff = sum(nks[:k])     # output row offset
                ob = outp.tile([P, tk * UC], fp16)
                for j in range(tk):
                    lhs = vmov[0:D, coff + j * P : coff + (j + 1) * P]
                    po = mmp.tile([P, UC], f32)
                    for h in range(NH):
                        hs = slice(h * NBLK, (h + 1) * NBLK)
                        nc.tensor.matmul(
                            po[:, hs],
                            lhsT=lhs,
                            rhs=uhi[:, hs],
                            start=True,
                            stop=True,
                        )
                    rows = P if j < fk else rk
                    osl = slice(j * UC, (j + 1) * UC)
                    eng = nc.vector if cp % 2 == 0 else nc.scalar
                    cp += 1
                    if eng is nc.vector:
                        eng.tensor_copy(out=ob[0:rows, osl], in_=po[0:rows, :])
                    else:
                        eng.copy(out=ob[0:rows, osl], in_=po[0:rows, :])
                if fk:
                    dst = out_dram[rowoff : rowoff + fk * P, :].rearrange(
                        "(a p) n -> p a n", p=P
                    )
                    src = ob[:, 0 : fk * UC].rearrange("p (a n) -> p a n", n=UC)
                    nc.sync.dma_start(out=dst, in_=src)
                if rk:
                    nc.sync.dma_start(
                        out=out_dram[rowoff + fk * P : rowoff + fk * P + rk, :],
                        in_=ob[0:rk, fk * UC : (fk + 1) * UC],
                    )
    nc.finalize()
    return nc


def _prep_items(ids):
    """Bucket item ids by static table ranges. Returns (mks, nks, perm,
    idx16 array [128, sum(mks)//16])."""
    b = ids // BWIDTH
    perm = np.argsort(b, kind="stable")
    sids = ids[perm]
    sb = b[perm]
    nks, chunks = [], []
    for k in range(NBUCKET):
        sel = sids[sb == k]
        n = len(sel)
        m = _ceil128(max(n, 1))
        loc = np.zeros(m, dtype=np.int16)
        loc[:n] = (sel - k * BWIDTH).astype(np.int16)
        nks.append(n)
        chunks.append(loc)
    mks = tuple(len(c) for c in chunks)
    idx16 = np.concatenate(chunks)
    wrapped = idx16.reshape(-1, 16).T            # [16, sum(mks)//16]
    rep = np.tile(wrapped, (8, 1))               # [128, ...]
    return mks, tuple(nks), perm, np.ascontiguousarray(rep)


def kernel(user_hiddens, item_hiddens, user_ids, item_ids, **_):
    user_hiddens = np.ascontiguousarray(user_hiddens, dtype=np.float32)
    item_hiddens = np.asarray(item_hiddens, dtype=np.float32)
    user_ids = np.asarray(user_ids)
    item_ids = np.asarray(item_ids)

    item_dup = np.empty((NUM_ITEMS, E), dtype=ml_dtypes.bfloat16)
    item_dup[:, 0:D] = item_hiddens
    item_dup[:, D:E] = item_dup[:, 0:D]

    preps = []
    for ci in range(RI):
        ids = item_ids[ci * IC : (ci + 1) * IC].astype(np.int64)
        preps.append(_prep_items(ids))
    # one program per distinct bucket-shape pair; RI=2 shapes must match to
    # stay SPMD, so build with the max per-bucket sizes padded identically
    mks = tuple(
        max(preps[ci][0][k] for ci in range(RI)) for k in range(NBUCKET)
    )
    if any(preps[ci][0] != mks for ci in range(RI)):
        # rebuild idx arrays padded to common shape
        new_preps = []
        for ci in range(RI):
            ids = item_ids[ci * IC : (ci + 1) * IC].astype(np.int64)
            b = ids // BWIDTH
            perm = np.argsort(b, kind="stable")
            sids = ids[perm]
            sb = b[perm]
            nks, chunks = [], []
            for k in range(NBUCKET):
                sel = sids[sb == k]
                n = len(sel)
                loc = np.zeros(mks[k], dtype=np.int16)
                loc[:n] = (sel - k * BWIDTH).astype(np.int16)
                nks.append(n)
                chunks.append(loc)
            idx16 = np.concatenate(chunks)
            wrapped = idx16.reshape(-1, 16).T
            rep = np.tile(wrapped, (8, 1))
            new_preps.append((mks, tuple(nks), perm, np.ascontiguousarray(rep)))
        preps = new_preps

    # nks may differ between the two item groups; the partial-row DMA
    # bookkeeping is per-core program state, so SPMD requires equal nks
    # too. Use per-ci programs only if they differ; normally the harness
    # inputs give one shape. Fall back: treat all rows as real (nks=mks)
    # and DMA pad rows into a padded out tensor? Simpler: require equal.
    nks0 = preps[0][1]
    same = all(preps[ci][1] == nks0 for ci in range(RI))
    if not same:
        # pad nks to mks: copy/DMA everything (incl. garbage pad rows) into
        # a padded output; host drops pads. Costs a few % extra copies/DMA.
        nks_use = mks
    else:
        nks_use = nks0

    ckey = (mks, nks_use)
    if ckey not in _cache:
        _cache.clear()
        _cache[ckey] = _build(mks, nks_use)
    nc = _cache[ckey]

    in_maps = []
    for c in range(N_CORES):
        cu, ci = divmod(c, RI)
        uc = user_ids[cu * UC : (cu + 1) * UC]
        uids_t = np.ascontiguousarray(uc.astype(np.int32).reshape(UT, P).T)
        in_maps.append(
            {
                "user_table": user_hiddens,
                "item_dup": item_dup,
                "uids": uids_t,
                "iidx": preps[ci][3],
            }
        )

    res = run_bass_kernel_spmd(nc, in_maps, list(range(N_CORES)))
    out = np.empty((BU, BI), dtype=np.float32)
    for c in range(N_CORES):
        cu, ci = divmod(c, RI)
        mks_ci, nks_ci, perm, _ = preps[ci]
        block = res.results[c]["out"]  # [IC, UC] fp16, bucket order rows
        if nks_use is mks:
            # padded mode: real rows are the first nks_ci[k] of each
            # bucket's padded region
            sel = []
            off = 0
            for k in range(NBUCKET):
                sel.extend(range(off, off + nks_ci[k]))
                off += mks[k]
            block = block[sel, :]
        out[
            cu * UC : (cu + 1) * UC, ci * IC + perm
        ] = block.T.astype(np.float32)
    return out
